# revision 6
# baseline (speedup 1.0000x reference)
"""Abeles matrix (neutron reflectivity) kernel for 8 Trainium2 NeuronCores.

Algorithm (per point (b,q), L=64 layers):
  X = (q/2)^2 - 4pi*(sld_l - sld_0)*1e-6, Y = 4pi*1e-9, R = sqrt(X^2+Y^2)
  k_l = A - iB: A = sqrt((R+X)/2) (stable for X>=0, clamped),
  B = min(sqrt((R-X)/2), (Y/2)/A) -- the min auto-selects the stable branch
  (clamp guarantees A_clamped <= A_true for X<0 so (Y/2)/A >= B_true there).
  r = (RDIF + 2i*CI0)/|S|^2 * exp(-2 s^2 PR)   [small-angle WI term dropped]
  E = exp(-2 t B) * (cos 2tA - i sin 2tA),  cos(x) = sin(pi/2 - |x|)
  scaled transfer recurrence (fp16): u0' = u0 + r*u1 ; u1' = E*(r*u0 + u1)
  out = |u1/u0|^2

fp16 bulk products are scaled by SC=2^8 (A,B scaled by SC) to avoid fp16
subnormal flush near the critical edge; the scale folds into ACT scale
factors, STT scalars and host-side param prep for free.

Sharding: pure data-parallel over batch, 32 rows of B=256 per core.
Per-core layout: 128 partitions = 32 b x 4 q-groups, 128 free = q within group.
Engine split: DVE = chain + fp16 2x bulk; ACT = transcendentals (table-set
grouped); GPSIMD = clamps, sums, scalar affine ops, negates.
"""
import sys
sys.path.insert(0, "/opt/trn_rl_repo")
import math
import numpy as np

import concourse.bass as bass
import concourse.mybir as mybir
from concourse import tile
from contextlib import ExitStack

AF = mybir.ActivationFunctionType
ALU = mybir.AluOpType
F32 = mybir.dt.float32
F16 = mybir.dt.float16
f32 = np.float32
f16 = np.float16

B, Q, L = 256, 512, 64
NCORES = 8
BL = B // NCORES           # 32 batch rows per core
P = 128                    # partitions
QF = 128                   # q elements per partition
CHUNK = 16                 # layers per chunk (4 chunks)

SC = 256.0                 # fp16 product scale (A,B scaled by SC)
ISC2 = f32(1.0 / (SC * SC))
YMAG = 4.0 * math.pi * 1e-9
Y2 = f32(YMAG * YMAG)
CLAMP = f32(4e-9)
LNYSC = f32(math.log(YMAG * SC / 2.0))
PIO2 = f32(np.pi / 2.0)
INV2PI = f32(1.0 / (2.0 * np.pi))
MAGIC = f32(1.5 * 2.0 ** 23)

# ---------------------------------------------------------------------------
# Toolchain workarounds for this walrus build:
# 1) InstDrain cannot carry sem waits -> re-emit as sync-engine wait_ge's.
# 2) TensorScalarPtr / Activation-with-AP-scale / CopyPredicated cannot carry
#    sem waits -> strip them onto same-engine wait_ge carrier instructions.
# ---------------------------------------------------------------------------
_PATCHED = False


def _install_patches():
    global _PATCHED
    if _PATCHED:
        return
    _PATCHED = True

    def _handles(tc):
        hm = {}
        for h in tc.sems.allocated().values():
            hm[h.name] = h
        return hm

    def _drain_and_barrier(self, tick_clock, wait_clock):
        nc = self.nc
        drain_inst = nc.sync.drain()
        wait_clock.add_sem_waits(
            drain_inst.ins, tile.ScopedClock({None: tick_clock.global_clock})
        )
        ii = drain_inst.ins
        si = ii.sync_info
        waits = list(si.on_wait) if si is not None else []
        if waits:
            ii.sync_info = mybir.SyncInfo(on_wait=[], on_update=list(si.on_update))
            hm = _handles(self)
            for w in waits:
                h = hm.get(w.ant_name)
                assert h is not None and w.wait_mode == "sem-ge-imm"
                nc.sync.wait_ge(h, w.wait_value)
        nc.all_engine_barrier()
        assert self.sems is not None
        popped = nc._tile_sem_poison_stack.pop()
        assert popped is self._sem_poison
        nc.clear_and_free_semaphores(list(self.sems.allocated().values()))
        nc.all_engine_barrier()

    tile.TileContext._drain_and_barrier = _drain_and_barrier

    _orig_commit = tile.TileContext._commit_instruction

    _KEEP1 = (mybir.InstTensorLoad, mybir.InstTensorSave, mybir.InstTensorCopy,
              mybir.InstTensorTensor)

    def _simple_aps(inst):
        # >2-dim APs lower to the S3S3D3-style structs with no wait slots
        try:
            for a in list(inst.ins) + list(inst.outs):
                ap = getattr(a, "ap", None)
                if ap is not None and len(ap) > 2:
                    return False
        except Exception:
            return False
        return True

    def _commit_instruction(self, inst, lazy_reg_writes=True):
        si = getattr(inst, "sync_info", None)
        if si is not None and si.on_wait:
            waits = list(si.on_wait)
            keep = []
            if isinstance(inst, _KEEP1) and _simple_aps(inst):
                # these structs tolerate one wait; strip the rest
                keep = waits[:1]
                waits = waits[1:]
            if waits:
                inst.sync_info = mybir.SyncInfo(on_wait=keep, on_update=list(si.on_update))
                hm = _handles(self)
                eng = self.nc.engines[inst.engine]
                for i in range(0, len(waits), 2):
                    grp = waits[i:i + 2]
                    h = hm.get(grp[0].ant_name)
                    assert h is not None and grp[0].wait_mode == "sem-ge-imm", grp
                    carrier = eng.wait_ge(h, grp[0].wait_value)
                    if len(grp) > 1:
                        csi = carrier.ins.sync_info
                        carrier.ins.sync_info = mybir.SyncInfo(
                            on_wait=list(grp),
                            on_update=list(csi.on_update) if csi else [])
        return _orig_commit(self, inst, lazy_reg_writes)

    tile.TileContext._commit_instruction = _commit_instruction


# ---------------------------------------------------------------------------
# Kernel builder (one NeuronCore program; SPMD across 8 cores)
# ---------------------------------------------------------------------------

def _build_kernel():
    _install_patches()
    nc = bass.Bass()

    d_qq = nc.declare_dram_parameter("qq", [P, QF], F32, isOutput=False)
    d_negc = nc.declare_dram_parameter("negc", [P, L + 1], F32, isOutput=False)
    d_s2m = nc.declare_dram_parameter("s2m16", [P, L], F16, isOutput=False)
    d_t2 = nc.declare_dram_parameter("t2", [P, L], F32, isOutput=False)
    d_m2t = nc.declare_dram_parameter("m2t16", [P, L], F16, isOutput=False)
    d_out = nc.declare_dram_parameter("out", [P, QF], F32, isOutput=True)

    with tile.TileContext(nc) as tc, ExitStack() as ctx:
        pool = ctx.enter_context(tc.tile_pool(name="sb", bufs=1))

        def tl(name, shape, dtype=F32, bufs=1):
            return pool.tile(shape, dtype, tag=name, name=name, bufs=bufs)

        # persistent inputs
        qq = tl("qq", [P, QF])
        negc = tl("negc", [P, L + 1])
        s2m16 = tl("s2m16", [P, L], F16)
        t2 = tl("t2", [P, L])
        m2t16 = tl("m2t16", [P, L], F16)
        nc.sync.dma_start(qq[:], d_qq[:])
        nc.sync.dma_start(negc[:], d_negc[:])
        nc.sync.dma_start(s2m16[:], d_s2m[:])
        nc.sync.dma_start(t2[:], d_t2[:])
        nc.sync.dma_start(m2t16[:], d_m2t[:])

        # constant bias vectors
        y2b = tl("y2b", [P, 1]);  nc.gpsimd.memset(y2b[:], float(Y2))
        lnyb = tl("lnyb", [P, 1]); nc.gpsimd.memset(lnyb[:], float(LNYSC))
        pio2b = tl("pio2b", [P, 1]); nc.gpsimd.memset(pio2b[:], float(PIO2))

        # u state (ping-pong quads: blocks [u0r | u0i | u1r | u1i]), fp16
        ucur = tl("uq_a", [P, 4 * QF], F16)
        unew = tl("uq_b", [P, 4 * QF], F16)
        nc.gpsimd.memset(ucur[:, 0:QF], 1.0)
        nc.gpsimd.memset(ucur[:, QF:], 0.0)
        A4 = tl("A4", [P, 4 * QF], F16); B4 = tl("B4", [P, 4 * QF], F16)
        T4 = tl("T4", [P, 4 * QF], F16)
        E4 = tl("E4", [P, 2 * QF], F16); E5 = tl("E5", [P, 2 * QF], F16)

        def pbc(param, lv0, n_l):
            # [P, n_l, QF] view of param[:, lv0:lv0+n_l], value broadcast over q
            return param[:, lv0:lv0 + n_l].rearrange("p (l n) -> p l n", n=1).broadcast_to([P, n_l, QF])

        def blk(t_, n_l):
            return t_[:, :n_l * QF].rearrange("p (l n) -> p l n", n=QF)

        prev_act = None   # serialization spine for ACT table-set ordering

        def act(out, in_, func, new_group=False, **kw):
            nonlocal prev_act
            i = nc.scalar.activation(out, in_, func, **kw)
            if prev_act is not None:
                bass._add_dep_helper(i.ins, prev_act.ins, sync=False,
                                     reason="act-table-order")
            prev_act = i
            return i

        starts = list(range(0, L, CHUNK))
        chunks = [(s0_, min(CHUNK, L - s0_)) for s0_ in starts]

        for l0, CL in reversed(chunks):
            cw = CL * QF
            cw1 = (CL + 1) * QF
            # ---- levels: X, R, U1, U2, A, B -------------------------------
            X = tl("X", [P, (CHUNK + 1) * QF])    # X -> U2 (in-place)
            SQ = tl("SQ", [P, (CHUNK + 1) * QF])  # SQ -> U1 -> LNA
            R = tl("Rr", [P, (CHUNK + 1) * QF])
            A32 = tl("A32", [P, (CHUNK + 1) * QF])
            A16 = tl("A16", [P, (CHUNK + 1) * QF], F16)
            B16 = tl("B16", [P, (CHUNK + 1) * QF], F16)
            BX16 = tl("BX16", [P, (CHUNK + 1) * QF], F16)
            MSKB = tl("MSKB", [P, (CHUNK + 1) * QF], F16)
            qqbc = qq[:].rearrange("p (l n) -> p l n", l=1).broadcast_to([P, CL + 1, QF])
            nc.vector.tensor_add(blk(X, CL + 1), qqbc, pbc(negc, l0, CL + 1))
            act(SQ[:, :cw1], X[:, :cw1], AF.Square)                          # [sqrt set]
            act(R[:, :cw1], SQ[:, :cw1], AF.Sqrt, bias=y2b[:])
            U1 = SQ  # SQ dead
            U2 = X   # X dies after MSK + U2 below
            nc.vector.tensor_scalar(MSKB[:, :cw1], X[:, :cw1], 0.0, None, ALU.is_ge)
            nc.vector.tensor_add(U1[:, :cw1], R[:, :cw1], X[:, :cw1])
            nc.vector.tensor_sub(U2[:, :cw1], R[:, :cw1], X[:, :cw1])        # in-place on X
            # clamp: subtract noise floor, floor at 0 (guarantees A<=A_true for X<0)
            nc.gpsimd.tensor_scalar(U1[:, :cw1], U1[:, :cw1], float(CLAMP), 0.0,
                                    ALU.subtract, ALU.max)
            nc.gpsimd.tensor_scalar(U2[:, :cw1], U2[:, :cw1], float(CLAMP), 0.0,
                                    ALU.subtract, ALU.max)
            act(A32[:, :cw1], U1[:, :cw1], AF.Sqrt, bias=0.0, scale=0.5)
            act(A16[:, :cw1], U1[:, :cw1], AF.Sqrt, bias=0.0, scale=float(0.5 * SC * SC))
            act(B16[:, :cw1], U2[:, :cw1], AF.Sqrt, bias=0.0, scale=float(0.5 * SC * SC))
            # [logexp set] Bx = (Y*SC/2)/A  (stable for X>=0); merged additively:
            # B = Braw + (X>=0)*Bx  (Braw clamps to exactly 0 for X>=0)
            LNA = U1  # U1 dead after sqrts
            act(LNA[:, :cw1], A32[:, :cw1], AF.Ln)
            nc.gpsimd.tensor_scalar(LNA[:, :cw1], LNA[:, :cw1], -24.4, None, ALU.max)
            act(BX16[:, :cw1], LNA[:, :cw1], AF.Exp, bias=lnyb[:], scale=-1.0)
            nc.vector.tensor_mul(BX16[:, :cw1], MSKB[:, :cw1], BX16[:, :cw1])
            nc.vector.tensor_add(B16[:, :cw1], B16[:, :cw1], BX16[:, :cw1])

            ac, an = A16[:, 0:cw], A16[:, QF:cw1]
            bc, bn = B16[:, 0:cw], B16[:, QF:cw1]
            rc, rn = R[:, 0:cw], R[:, QF:cw1]

            # ---- layer quantities ----------------------------------------
            P1 = tl("P1", [P, CHUNK * QF], F16)
            P2 = tl("P2", [P, CHUNK * QF], F16)
            AB1 = tl("AB1", [P, CHUNK * QF], F16)
            AB2 = tl("AB2", [P, CHUNK * QF], F16)
            PR = tl("PR", [P, CHUNK * QF], F16)
            PPM = tl("PPM", [P, CHUNK * QF])        # PP -> MAG -> LNM (f32)
            CI0 = tl("CI0", [P, CHUNK * QF])        # f32
            RSUM = tl("RSUM", [P, CHUNK * QF])
            RDIF = tl("RDIF", [P, CHUNK * QF])
            ARG = tl("ARG", [P, CHUNK * QF], F16)
            ARG2 = tl("ARG2", [P, CHUNK * QF])      # -> RR (f32, in-place)
            TA2 = tl("TA2", [P, CHUNK * QF])        # -> TA2r in-place
            FT = tl("FT", [P, CHUNK * QF])          # Ft -> KK in-place -> AbsT
            TB2 = tl("TB2", [P, CHUNK * QF], F16)
            EE = tl("EE", [P, CHUNK * QF], F16)
            S2T = tl("S2T", [P, CHUNK * QF], F16)
            C2T = tl("C2T", [P, CHUNK * QF], F16)
            RRE = tl("RRE", [P, CHUNK * QF], F16, bufs=2)
            RIMPM = tl("RIMPM", [P, 2 * CHUNK * QF], F16, bufs=2)
            ERE = tl("ERE", [P, CHUNK * QF], F16, bufs=2)
            EIMPM = tl("EIMPM", [P, 2 * CHUNK * QF], F16, bufs=2)
            rimv = RIMPM[:, :2 * cw].rearrange("p (l t n) -> p l t n", t=2, n=QF)
            eimv = EIMPM[:, :2 * cw].rearrange("p (l t n) -> p l t n", t=2, n=QF)

            nc.vector.tensor_mul(P1[:, :cw], ac, an)
            nc.vector.tensor_mul(P2[:, :cw], bc, bn)
            nc.vector.tensor_mul(AB1[:, :cw], ac, bn)
            nc.vector.tensor_mul(AB2[:, :cw], bc, an)
            nc.vector.tensor_sub(PR[:, :cw], P1[:, :cw], P2[:, :cw])
            nc.vector.tensor_sub(CI0[:, :cw], AB1[:, :cw], AB2[:, :cw])
            nc.gpsimd.tensor_add(PPM[:, :cw], P1[:, :cw], P2[:, :cw])
            nc.gpsimd.tensor_add(RSUM[:, :cw], rc, rn)
            nc.gpsimd.tensor_sub(RDIF[:, :cw], rc, rn)
            # MAG = 2/SC^2 * PP + RSUM  (in-place over PP; Pool rejects STT)
            nc.gpsimd.tensor_scalar(PPM[:, :cw], PPM[:, :cw], float(2.0 * ISC2), None,
                                    ALU.mult)
            nc.gpsimd.tensor_add(PPM[:, :cw], PPM[:, :cw], RSUM[:, :cw])
            act(PPM[:, :cw], PPM[:, :cw], AF.Ln)                              # LNM in-place
            nc.vector.tensor_mul(blk(ARG, CL), blk(PR, CL), pbc(s2m16, l0, CL))
            nc.gpsimd.tensor_sub(ARG2[:, :cw], ARG[:, :cw], PPM[:, :cw])      # mixed f16/f32
            act(ARG2[:, :cw], ARG2[:, :cw], AF.Exp)                           # RR in-place f32
            nc.vector.tensor_mul(RRE[:, :cw], RDIF[:, :cw], ARG2[:, :cw])     # RATRE -> fp16
            # RATIM -> rimv slot1 (+); slot0 = negated
            nc.vector.scalar_tensor_tensor(rimv[:, :, 1, :], blk(CI0, CL),
                                           float(2.0 * ISC2), blk(ARG2, CL),
                                           ALU.mult, ALU.mult)
            nc.gpsimd.tensor_scalar(rimv[:, :, 0, :], rimv[:, :, 1, :], -1.0, None,
                                    ALU.mult)
            # ---- E = exp(-2tB) * (cos 2tA - i sin 2tA) --------------------
            nc.vector.tensor_mul(blk(TA2, CL), blk(A32, CL), pbc(t2, l0, CL))
            nc.vector.tensor_mul(blk(TB2, CL), blk(B16, CL), pbc(m2t16, l0, CL))
            nc.gpsimd.tensor_scalar(FT[:, :cw], TA2[:, :cw], float(INV2PI),
                                    float(MAGIC), ALU.mult, ALU.add)
            nc.gpsimd.tensor_scalar(FT[:, :cw], FT[:, :cw], float(MAGIC), None,
                                    ALU.subtract)                              # KK in-place
            nc.vector.scalar_tensor_tensor(TA2[:, :cw], FT[:, :cw],
                                           float(-2.0 * np.pi), TA2[:, :cw],
                                           ALU.mult, ALU.add)                  # TA2r in-place
            act(EE[:, :cw], TB2[:, :cw], AF.Exp)
            act(FT[:, :cw], TA2[:, :cw], AF.Abs)                               # AbsT (FT dead)
            act(S2T[:, :cw], TA2[:, :cw], AF.Sin)                              # [trig set]
            act(C2T[:, :cw], FT[:, :cw], AF.Sin, bias=pio2b[:], scale=-1.0)
            nc.vector.tensor_mul(ERE[:, :cw], EE[:, :cw], C2T[:, :cw])
            nc.vector.tensor_mul(eimv[:, :, 0, :], blk(EE, CL), blk(S2T, CL))
            nc.gpsimd.tensor_scalar(eimv[:, :, 1, :], eimv[:, :, 0, :], -1.0, None,
                                    ALU.mult)

            # ---- sequential update over layers (descending), fp16 ---------
            for j in range(CL - 1, -1, -1):
                sl = slice(j * QF, (j + 1) * QF)
                sl2 = slice(j * 2 * QF, (j + 1) * 2 * QF)
                U = ucur; Vq = unew
                U22 = U[:].rearrange("p (a b n) -> p a b n", a=2, b=2)
                u_rot = U22[:, ::-1, :, :]          # [u1r,u1i,u0r,u0i]
                u_rev = U22[:, ::-1, ::-1, :]       # [u1i,u1r,u0i,u0r]
                rre4 = RRE[:, sl].rearrange("p (a b n) -> p a b n", a=1, b=1).broadcast_to([P, 2, 2, QF])
                rim4 = RIMPM[:, sl2].rearrange("p (a t n) -> p a t n", a=1, t=2).broadcast_to([P, 2, 2, QF])
                nc.vector.tensor_mul(A4[:].rearrange("p (a b n) -> p a b n", a=2, b=2), u_rot, rre4)
                nc.vector.tensor_mul(B4[:].rearrange("p (a b n) -> p a b n", a=2, b=2), u_rev, rim4)
                nc.vector.tensor_add(T4[:], U[:], A4[:])
                nc.vector.tensor_add(Vq[:], T4[:], B4[:])   # [nu0r, nu0i, p1r, p1i]
                p1 = Vq[:, 2 * QF:4 * QF]
                p12 = p1.rearrange("p (b n) -> p b n", b=2)
                p1sw = p12[:, ::-1, :]
                ere2 = ERE[:, sl].rearrange("p (b n) -> p b n", b=1).broadcast_to([P, 2, QF])
                nc.vector.tensor_mul(E4[:].rearrange("p (b n) -> p b n", b=2), p12, ere2)
                nc.vector.tensor_mul(E5[:].rearrange("p (b n) -> p b n", b=2), p1sw,
                                     EIMPM[:, sl2].rearrange("p (b n) -> p b n", b=2))
                nc.vector.tensor_add(Vq[:, 2 * QF:4 * QF], E4[:], E5[:])
                ucur, unew = unew, ucur

        # ---- epilogue: out = |u1/u0|^2 -----------------------------------
        u0r, u0i = ucur[:, 0:QF], ucur[:, QF:2 * QF]
        u1r, u1i = ucur[:, 2 * QF:3 * QF], ucur[:, 3 * QF:4 * QF]
        d1 = tl("q1", [P, QF]); d2 = tl("q2", [P, QF]); d3 = tl("q3", [P, QF])
        act(d1[:], u0r, AF.Square)
        act(d2[:], u0i, AF.Square)
        nc.vector.tensor_add(d1[:], d1[:], d2[:])      # |u0|^2
        act(d2[:], d1[:], AF.Ln)
        act(d1[:], d2[:], AF.Exp, bias=0.0, scale=-1.0)  # 1/|u0|^2
        nc.vector.tensor_mul(d2[:], u1r, u0r)
        nc.vector.tensor_mul(d3[:], u1i, u0i)
        nc.vector.tensor_add(d2[:], d2[:], d3[:])
        nc.vector.tensor_mul(d2[:], d2[:], d1[:])      # qr
        OUT = tl("OUT", [P, QF])
        nc.vector.tensor_mul(d3[:], u1i, u0r)
        qi2 = tl("q4", [P, QF])
        nc.vector.tensor_mul(qi2[:], u1r, u0i)
        nc.vector.tensor_sub(d3[:], d3[:], qi2[:])
        nc.vector.tensor_mul(d3[:], d3[:], d1[:])      # qi
        act(d2[:], d2[:], AF.Square)
        act(d3[:], d3[:], AF.Square)
        nc.vector.tensor_add(OUT[:], d2[:], d3[:])
        nc.sync.dma_start(d_out[:], OUT[:])

    return nc


_NC_CACHE = None


def _get_nc():
    global _NC_CACHE
    if _NC_CACHE is None:
        _NC_CACHE = _build_kernel()
    return _NC_CACHE


def _prep_core_inputs(q, thickness, roughness, sld):
    """Host-side O(B*(Q+L)) prep; returns per-core input dicts."""
    q = q.astype(f32); th = thickness.astype(f32)
    rg = roughness.astype(f32); sld = sld.astype(f32)
    amb = sld[:, 0:1]
    negc64 = -(4.0 * math.pi * 1e-6) * (sld.astype(np.float64) - amb.astype(np.float64))
    negc = negc64.astype(f32)                       # [B, L+1]
    s2m16 = (-2.0 * rg * rg / (SC * SC)).astype(f16)   # scale folded
    t2 = (2.0 * th).astype(f32)
    m2t16 = (-2.0 * th / SC).astype(f16)               # scale folded
    qq = ((q * f32(0.5)) ** 2).astype(f32)          # [B, Q]

    def rep4(arr):  # [BL, K] -> [128, K] (each row repeated 4x)
        return np.repeat(arr, 4, axis=0).copy()

    in_maps = []
    for c in range(NCORES):
        bs = slice(c * BL, (c + 1) * BL)
        in_maps.append({
            "qq": qq[bs].reshape(P, QF).copy(),
            "negc": rep4(negc[bs]),
            "s2m16": rep4(s2m16[bs]),
            "t2": rep4(t2[bs]),
            "m2t16": rep4(m2t16[bs]),
        })
    return in_maps


def run(q, thickness, roughness, sld, trace=False, **trace_kwargs):
    from concourse.bass_utils import run_bass_kernel_spmd
    nc = _get_nc()
    in_maps = _prep_core_inputs(q, thickness, roughness, sld)
    res = run_bass_kernel_spmd(nc, in_maps, core_ids=list(range(NCORES)),
                               trace=trace, **trace_kwargs)
    out = np.empty((B, Q), f32)
    for c in range(NCORES):
        out[c * BL:(c + 1) * BL] = res.results[c]["out"].reshape(BL, Q)
    return out, res


def kernel(q, thickness, roughness, sld):
    out, _ = run(q, thickness, roughness, sld)
    return out


# revision 10
# speedup vs baseline: 3.2343x; 3.2343x over previous
"""Abeles matrix (neutron reflectivity) kernel for 8 Trainium2 NeuronCores.

Algorithm (per point (b,q), L=64 layers):
  X = (q/2)^2 - 4pi*(sld_l - sld_0)*1e-6, Y = 4pi*1e-9, R = sqrt(X^2+Y^2)
  k_l = A - iB: A = sqrt((R+X)/2) (stable for X>=0, clamped),
  B = min(sqrt((R-X)/2), (Y/2)/A) -- the min auto-selects the stable branch
  (clamp guarantees A_clamped <= A_true for X<0 so (Y/2)/A >= B_true there).
  r = (RDIF + 2i*CI0)/|S|^2 * exp(-2 s^2 PR)   [small-angle WI term dropped]
  E = exp(-2 t B) * (cos 2tA - i sin 2tA),  cos(x) = sin(pi/2 - |x|)
  scaled transfer recurrence (fp16): u0' = u0 + r*u1 ; u1' = E*(r*u0 + u1)
  out = |u1/u0|^2

fp16 bulk products are scaled by SC=2^8 (A,B scaled by SC) to avoid fp16
subnormal flush near the critical edge; the scale folds into ACT scale
factors, STT scalars and host-side param prep for free.

Sharding: pure data-parallel over batch, 32 rows of B=256 per core.
Per-core layout: 128 partitions = 32 b x 4 q-groups, 128 free = q within group.
Engine split: DVE = chain + fp16 2x bulk; ACT = transcendentals (table-set
grouped); GPSIMD = clamps, sums, scalar affine ops, negates.
"""
import sys
sys.path.insert(0, "/opt/trn_rl_repo")
import math
import numpy as np

import concourse.bass as bass
import concourse.mybir as mybir
from concourse import tile
from contextlib import ExitStack

AF = mybir.ActivationFunctionType
ALU = mybir.AluOpType
F32 = mybir.dt.float32
F16 = mybir.dt.float16
f32 = np.float32
f16 = np.float16

B, Q, L = 256, 512, 64
NCORES = 8
BL = B // NCORES           # 32 batch rows per core
P = 128                    # partitions
QF = 128                   # q elements per partition
CHUNK = 16                 # layers per chunk (4 chunks)

SC = 256.0                 # fp16 product scale (A,B scaled by SC)
ISC2 = f32(1.0 / (SC * SC))
YMAG = 4.0 * math.pi * 1e-9
Y2 = f32(YMAG * YMAG)
CLAMP = f32(4e-9)
LNYSC = f32(math.log(YMAG * SC / 2.0))
SC4 = f32(SC * SC * SC * SC)
ABIAS = f32(2e-9)
PIO2 = f32(np.pi / 2.0)
INV2PI = f32(1.0 / (2.0 * np.pi))
MAGIC = f32(1.5 * 2.0 ** 23)

# ---------------------------------------------------------------------------
# Toolchain workarounds for this walrus build:
# 1) InstDrain cannot carry sem waits -> re-emit as sync-engine wait_ge's.
# 2) TensorScalarPtr / Activation-with-AP-scale / CopyPredicated cannot carry
#    sem waits -> strip them onto same-engine wait_ge carrier instructions.
# ---------------------------------------------------------------------------
_PATCHED = False


def _install_patches():
    global _PATCHED
    if _PATCHED:
        return
    _PATCHED = True

    def _handles(tc):
        hm = {}
        for h in tc.sems.allocated().values():
            hm[h.name] = h
        return hm

    def _drain_and_barrier(self, tick_clock, wait_clock):
        nc = self.nc
        drain_inst = nc.sync.drain()
        wait_clock.add_sem_waits(
            drain_inst.ins, tile.ScopedClock({None: tick_clock.global_clock})
        )
        ii = drain_inst.ins
        si = ii.sync_info
        waits = list(si.on_wait) if si is not None else []
        if waits:
            ii.sync_info = mybir.SyncInfo(on_wait=[], on_update=list(si.on_update))
            hm = _handles(self)
            for w in waits:
                h = hm.get(w.ant_name)
                assert h is not None and w.wait_mode == "sem-ge-imm"
                nc.sync.wait_ge(h, w.wait_value)
        nc.all_engine_barrier()
        assert self.sems is not None
        popped = nc._tile_sem_poison_stack.pop()
        assert popped is self._sem_poison
        nc.clear_and_free_semaphores(list(self.sems.allocated().values()))
        nc.all_engine_barrier()

    tile.TileContext._drain_and_barrier = _drain_and_barrier

    _orig_commit = tile.TileContext._commit_instruction

    _KEEP1 = (mybir.InstTensorLoad, mybir.InstTensorSave, mybir.InstTensorCopy,
              mybir.InstTensorTensor)

    def _simple_aps(inst):
        # >2-dim APs lower to the S3S3D3-style structs with no wait slots
        try:
            for a in list(inst.ins) + list(inst.outs):
                ap = getattr(a, "ap", None)
                if ap is not None and len(ap) > 2:
                    return False
        except Exception:
            return False
        return True

    def _commit_instruction(self, inst, lazy_reg_writes=True):
        si = getattr(inst, "sync_info", None)
        if si is not None and si.on_wait:
            waits = list(si.on_wait)
            keep = []
            if isinstance(inst, _KEEP1) and _simple_aps(inst):
                # these structs tolerate one wait; strip the rest
                keep = waits[:1]
                waits = waits[1:]
            if waits:
                inst.sync_info = mybir.SyncInfo(on_wait=keep, on_update=list(si.on_update))
                hm = _handles(self)
                eng = self.nc.engines[inst.engine]
                for i in range(0, len(waits), 2):
                    grp = waits[i:i + 2]
                    h = hm.get(grp[0].ant_name)
                    assert h is not None and grp[0].wait_mode == "sem-ge-imm", grp
                    carrier = eng.wait_ge(h, grp[0].wait_value)
                    if len(grp) > 1:
                        csi = carrier.ins.sync_info
                        carrier.ins.sync_info = mybir.SyncInfo(
                            on_wait=list(grp),
                            on_update=list(csi.on_update) if csi else [])
        return _orig_commit(self, inst, lazy_reg_writes)

    tile.TileContext._commit_instruction = _commit_instruction


# ---------------------------------------------------------------------------
# Kernel builder (one NeuronCore program; SPMD across 8 cores)
# ---------------------------------------------------------------------------

def _build_kernel():
    _install_patches()
    nc = bass.Bass()

    d_qq = nc.declare_dram_parameter("qq", [P, QF], F32, isOutput=False)
    d_negc = nc.declare_dram_parameter("negc", [P, L + 1], F32, isOutput=False)
    d_s2m = nc.declare_dram_parameter("s2m16", [P, L], F16, isOutput=False)
    d_t2 = nc.declare_dram_parameter("t2", [P, L], F32, isOutput=False)
    d_m2t = nc.declare_dram_parameter("m2t16", [P, L], F16, isOutput=False)
    d_out = nc.declare_dram_parameter("out", [P, QF], F32, isOutput=True)

    with tile.TileContext(nc) as tc, ExitStack() as ctx:
        pool = ctx.enter_context(tc.tile_pool(name="sb", bufs=1))

        def tl(name, shape, dtype=F32, bufs=1):
            return pool.tile(shape, dtype, tag=name, name=name, bufs=bufs)

        # persistent inputs
        qq = tl("qq", [P, QF])
        negc = tl("negc", [P, L + 1])
        s2m16 = tl("s2m16", [P, L], F16)
        t2 = tl("t2", [P, L])
        m2t16 = tl("m2t16", [P, L], F16)
        nc.sync.dma_start(qq[:], d_qq[:])
        nc.sync.dma_start(negc[:], d_negc[:])
        nc.sync.dma_start(s2m16[:], d_s2m[:])
        nc.sync.dma_start(t2[:], d_t2[:])
        nc.sync.dma_start(m2t16[:], d_m2t[:])

        # constant bias vectors
        y2b = tl("y2b", [P, 1]);  nc.gpsimd.memset(y2b[:], float(Y2))
        nls2b = tl("nls2b", [P, 1]); nc.gpsimd.memset(nls2b[:], float(-math.log(SC * SC)))
        lnyb = tl("lnyb", [P, 1]); nc.gpsimd.memset(lnyb[:], float(LNYSC))
        pio2b = tl("pio2b", [P, 1]); nc.gpsimd.memset(pio2b[:], float(PIO2))

        # u state (ping-pong quads: blocks [u0r | u0i | u1r | u1i]), fp16
        ucur = tl("uq_a", [P, 4 * QF], F16)
        unew = tl("uq_b", [P, 4 * QF], F16)
        nc.gpsimd.memset(ucur[:, 0:QF], 1.0)
        nc.gpsimd.memset(ucur[:, QF:], 0.0)
        A4 = tl("A4", [P, 4 * QF], F16); B4 = tl("B4", [P, 4 * QF], F16)
        T4 = tl("T4", [P, 4 * QF], F16)
        E4 = tl("E4", [P, 2 * QF], F16); E5 = tl("E5", [P, 2 * QF], F16)

        def pbc(param, lv0, n_l):
            # [P, n_l, QF] view of param[:, lv0:lv0+n_l], value broadcast over q
            return param[:, lv0:lv0 + n_l].rearrange("p (l n) -> p l n", n=1).broadcast_to([P, n_l, QF])

        def blk(t_, n_l):
            return t_[:, :n_l * QF].rearrange("p (l n) -> p l n", n=QF)

        prev_act = None   # serialization spine for ACT table-set ordering

        def act(out, in_, func, new_group=False, **kw):
            nonlocal prev_act
            i = nc.scalar.activation(out, in_, func, **kw)
            if prev_act is not None:
                bass._add_dep_helper(i.ins, prev_act.ins, sync=False,
                                     reason="act-table-order")
            prev_act = i
            return i

        starts = list(range(0, L, CHUNK))
        chunks = [(s0_, min(CHUNK, L - s0_)) for s0_ in starts]

        for l0, CL in reversed(chunks):
            cw = CL * QF
            cw1 = (CL + 1) * QF
            # ---- levels: X, R, U1, U2, A, B -------------------------------
            X = tl("X", [P, (CHUNK + 1) * QF])    # X -> U2 (in-place)
            SQ = tl("SQ", [P, (CHUNK + 1) * QF])  # SQ -> U1 -> LNA
            R = tl("Rr", [P, (CHUNK + 1) * QF])
            A32 = tl("A32", [P, (CHUNK + 1) * QF])
            A16 = tl("A16", [P, (CHUNK + 1) * QF], F16)
            B16 = tl("B16", [P, (CHUNK + 1) * QF], F16)
            BX16 = tl("BX16", [P, (CHUNK + 1) * QF], F16)
            MSKB = tl("MSKB", [P, (CHUNK + 1) * QF], F16)
            qqbc = qq[:].rearrange("p (l n) -> p l n", l=1).broadcast_to([P, CL + 1, QF])
            nc.vector.tensor_add(blk(X, CL + 1), qqbc, pbc(negc, l0, CL + 1))
            act(SQ[:, :cw1], X[:, :cw1], AF.Square)                          # [sqrt set]
            act(R[:, :cw1], SQ[:, :cw1], AF.Sqrt, bias=y2b[:])
            U1 = SQ  # SQ dead
            U2 = X   # X dies after MSK + U2 below
            nc.vector.tensor_scalar(MSKB[:, :cw1], X[:, :cw1], 0.0, None, ALU.is_ge)
            nc.vector.tensor_add(U1[:, :cw1], R[:, :cw1], X[:, :cw1])
            nc.vector.tensor_sub(U2[:, :cw1], R[:, :cw1], X[:, :cw1])        # in-place on X
            # clamps: subtract noise floor, floor at 0 (exact 0 for the dead branch)
            nc.vector.tensor_scalar(U1[:, :cw1], U1[:, :cw1], float(CLAMP), 0.0,
                                    ALU.subtract, ALU.max)
            nc.vector.tensor_scalar(U2[:, :cw1], U2[:, :cw1], float(CLAMP), 0.0,
                                    ALU.subtract, ALU.max)
            act(A32[:, :cw1], U1[:, :cw1], AF.Sqrt, bias=0.0, scale=0.5)
            act(A16[:, :cw1], U1[:, :cw1], AF.Sqrt, bias=0.0, scale=float(0.5 * SC * SC))
            act(B16[:, :cw1], U2[:, :cw1], AF.Sqrt, bias=0.0, scale=float(0.5 * SC * SC))
            # [logexp set] Bx = (Y*SC/2)/A  (stable for X>=0); merged additively:
            # B = Braw + (X>=0)*min(Bx, 6e4)  (cap keeps 0*inf from making NaN)
            LNA = U1  # U1 dead after sqrts
            act(LNA[:, :cw1], A32[:, :cw1], AF.Ln)
            act(BX16[:, :cw1], LNA[:, :cw1], AF.Exp, bias=lnyb[:], scale=-1.0)
            nc.vector.tensor_scalar(BX16[:, :cw1], BX16[:, :cw1], 60000.0, None, ALU.min)
            nc.vector.tensor_mul(BX16[:, :cw1], MSKB[:, :cw1], BX16[:, :cw1])
            nc.vector.tensor_add(B16[:, :cw1], B16[:, :cw1], BX16[:, :cw1])

            ac, an = A16[:, 0:cw], A16[:, QF:cw1]
            bc, bn = B16[:, 0:cw], B16[:, QF:cw1]
            rc, rn = R[:, 0:cw], R[:, QF:cw1]

            # ---- layer quantities ----------------------------------------
            P1 = tl("P1", [P, CHUNK * QF], F16)     # P1 -> PR -> RR16
            P2 = tl("P2", [P, CHUNK * QF], F16)     # P2 -> ARG
            AB1 = tl("AB1", [P, CHUNK * QF], F16)   # AB1 -> CI0
            AB2 = tl("AB2", [P, CHUNK * QF], F16)   # AB2 -> RATIM
            PP = tl("PP", [P, CHUNK * QF], F16)
            MAG = tl("MAG", [P, CHUNK * QF])        # MAG -> LNM (in-place)
            RSUM = tl("RSUM", [P, CHUNK * QF])      # RSUM -> ARG2
            RDIF = tl("RDIF", [P, CHUNK * QF])
            TA2 = tl("TA2", [P, CHUNK * QF])        # -> TA2r in-place
            FT = tl("FT", [P, CHUNK * QF])          # Ft -> KK in-place -> AbsT
            TB2 = tl("TB2", [P, CHUNK * QF], F16)
            EE = tl("EE", [P, CHUNK * QF], F16)
            S2T = tl("S2T", [P, CHUNK * QF], F16)
            C2T = tl("C2T", [P, CHUNK * QF], F16)
            RRE = tl("RRE", [P, CHUNK * QF], F16, bufs=2)
            RIMPM = tl("RIMPM", [P, 2 * CHUNK * QF], F16, bufs=2)
            ERE = tl("ERE", [P, CHUNK * QF], F16, bufs=2)
            EIMPM = tl("EIMPM", [P, 2 * CHUNK * QF], F16, bufs=2)
            rimv = RIMPM[:, :2 * cw].rearrange("p (l t n) -> p l t n", t=2, n=QF)
            eimv = EIMPM[:, :2 * cw].rearrange("p (l t n) -> p l t n", t=2, n=QF)

            nc.vector.tensor_mul(P1[:, :cw], ac, an)
            nc.vector.tensor_mul(P2[:, :cw], bc, bn)
            nc.vector.tensor_mul(AB1[:, :cw], ac, bn)
            nc.vector.tensor_mul(AB2[:, :cw], bc, an)
            nc.vector.tensor_add(PP[:, :cw], P1[:, :cw], P2[:, :cw])
            PR = P1   # in-place: P1 dead after PP
            nc.vector.tensor_sub(PR[:, :cw], P1[:, :cw], P2[:, :cw])
            CI0 = AB1  # in-place: AB1 dead after
            nc.vector.tensor_sub(CI0[:, :cw], AB1[:, :cw], AB2[:, :cw])
            nc.vector.tensor_add(RSUM[:, :cw], rc, rn)
            nc.vector.tensor_sub(RDIF[:, :cw], rc, rn)
            # MAG = |S|^2 unscaled f32; -ln(SC^2) folds into the Exp bias below
            nc.vector.scalar_tensor_tensor(MAG[:, :cw], PP[:, :cw], float(2.0 * ISC2),
                                           RSUM[:, :cw], ALU.mult, ALU.add)
            LNM = MAG
            act(LNM[:, :cw], MAG[:, :cw], AF.Ln)                              # in-place f32
            ARG = P2   # P2 dead after PP/PR
            nc.vector.tensor_mul(blk(ARG, CL), blk(PR, CL), pbc(s2m16, l0, CL))
            ARG2 = RSUM  # RSUM dead after MAG
            nc.vector.tensor_sub(ARG2[:, :cw], ARG[:, :cw], LNM[:, :cw])      # mixed -> f32
            RR16 = PR   # PR dead after ARG (fp16 slot)
            act(RR16[:, :cw], ARG2[:, :cw], AF.Exp, bias=nls2b[:])            # RR/SC^2 fp16
            nc.vector.scalar_tensor_tensor(RRE[:, :cw], RDIF[:, :cw], float(SC * SC),
                                           RR16[:, :cw], ALU.mult, ALU.mult)  # RATRE fp16
            # RATIM flat, then ACT copies into +/- slots of RIMPM
            RATIM = AB2  # AB2 dead after CI0
            nc.vector.scalar_tensor_tensor(RATIM[:, :cw], CI0[:, :cw], 2.0,
                                           RR16[:, :cw], ALU.mult, ALU.mult)
            act(rimv[:, :, 1, :], blk(RATIM, CL), AF.Copy, bias=0.0)
            act(rimv[:, :, 0, :], blk(RATIM, CL), AF.Copy, bias=0.0, scale=-1.0)
            # ---- E = exp(-2tB) * (cos 2tA - i sin 2tA) --------------------
            nc.vector.tensor_mul(blk(TA2, CL), blk(A32, CL), pbc(t2, l0, CL))
            nc.vector.tensor_mul(blk(TB2, CL), blk(B16, CL), pbc(m2t16, l0, CL))
            act(FT[:, :cw], TA2[:, :cw], AF.Copy, bias=float(MAGIC), scale=float(INV2PI))
            act(FT[:, :cw], FT[:, :cw], AF.Copy, bias=float(-MAGIC))             # KK in-place
            nc.vector.scalar_tensor_tensor(TA2[:, :cw], FT[:, :cw],
                                           float(-2.0 * np.pi), TA2[:, :cw],
                                           ALU.mult, ALU.add)                  # TA2r in-place
            act(EE[:, :cw], TB2[:, :cw], AF.Exp)
            act(FT[:, :cw], TA2[:, :cw], AF.Abs)                               # AbsT (FT dead)
            act(S2T[:, :cw], TA2[:, :cw], AF.Sin)                              # [trig set]
            act(C2T[:, :cw], FT[:, :cw], AF.Sin, bias=pio2b[:], scale=-1.0)
            nc.vector.tensor_mul(ERE[:, :cw], EE[:, :cw], C2T[:, :cw])
            nc.vector.tensor_mul(eimv[:, :, 0, :], blk(EE, CL), blk(S2T, CL))
            act(eimv[:, :, 1, :], eimv[:, :, 0, :], AF.Copy, bias=0.0, scale=-1.0)

            # ---- sequential update over layers (descending), fp16 ---------
            for j in range(CL - 1, -1, -1):
                sl = slice(j * QF, (j + 1) * QF)
                sl2 = slice(j * 2 * QF, (j + 1) * 2 * QF)
                U = ucur; Vq = unew
                U22 = U[:].rearrange("p (a b n) -> p a b n", a=2, b=2)
                u_rot = U22[:, ::-1, :, :]          # [u1r,u1i,u0r,u0i]
                u_rev = U22[:, ::-1, ::-1, :]       # [u1i,u1r,u0i,u0r]
                rre4 = RRE[:, sl].rearrange("p (a b n) -> p a b n", a=1, b=1).broadcast_to([P, 2, 2, QF])
                rim4 = RIMPM[:, sl2].rearrange("p (a t n) -> p a t n", a=1, t=2).broadcast_to([P, 2, 2, QF])
                nc.vector.tensor_mul(A4[:].rearrange("p (a b n) -> p a b n", a=2, b=2), u_rot, rre4)
                nc.vector.tensor_mul(B4[:].rearrange("p (a b n) -> p a b n", a=2, b=2), u_rev, rim4)
                nc.vector.tensor_add(T4[:], U[:], A4[:])
                nc.vector.tensor_add(Vq[:], T4[:], B4[:])   # [nu0r, nu0i, p1r, p1i]
                p1 = Vq[:, 2 * QF:4 * QF]
                p12 = p1.rearrange("p (b n) -> p b n", b=2)
                p1sw = p12[:, ::-1, :]
                ere2 = ERE[:, sl].rearrange("p (b n) -> p b n", b=1).broadcast_to([P, 2, QF])
                nc.vector.tensor_mul(E4[:].rearrange("p (b n) -> p b n", b=2), p12, ere2)
                nc.vector.tensor_mul(E5[:].rearrange("p (b n) -> p b n", b=2), p1sw,
                                     EIMPM[:, sl2].rearrange("p (b n) -> p b n", b=2))
                nc.vector.tensor_add(Vq[:, 2 * QF:4 * QF], E4[:], E5[:])
                ucur, unew = unew, ucur

        # ---- epilogue: out = |u1/u0|^2 -----------------------------------
        u0r, u0i = ucur[:, 0:QF], ucur[:, QF:2 * QF]
        u1r, u1i = ucur[:, 2 * QF:3 * QF], ucur[:, 3 * QF:4 * QF]
        d1 = tl("q1", [P, QF]); d2 = tl("q2", [P, QF]); d3 = tl("q3", [P, QF])
        act(d1[:], u0r, AF.Square)
        act(d2[:], u0i, AF.Square)
        nc.vector.tensor_add(d1[:], d1[:], d2[:])      # |u0|^2
        act(d2[:], d1[:], AF.Ln)
        act(d1[:], d2[:], AF.Exp, bias=0.0, scale=-1.0)  # 1/|u0|^2
        nc.vector.tensor_mul(d2[:], u1r, u0r)
        nc.vector.tensor_mul(d3[:], u1i, u0i)
        nc.vector.tensor_add(d2[:], d2[:], d3[:])
        nc.vector.tensor_mul(d2[:], d2[:], d1[:])      # qr
        OUT = tl("OUT", [P, QF])
        nc.vector.tensor_mul(d3[:], u1i, u0r)
        qi2 = tl("q4", [P, QF])
        nc.vector.tensor_mul(qi2[:], u1r, u0i)
        nc.vector.tensor_sub(d3[:], d3[:], qi2[:])
        nc.vector.tensor_mul(d3[:], d3[:], d1[:])      # qi
        act(d2[:], d2[:], AF.Square)
        act(d3[:], d3[:], AF.Square)
        nc.vector.tensor_add(OUT[:], d2[:], d3[:])
        nc.sync.dma_start(d_out[:], OUT[:])

    return nc


_NC_CACHE = None


def _get_nc():
    global _NC_CACHE
    if _NC_CACHE is None:
        _NC_CACHE = _build_kernel()
    return _NC_CACHE


def _prep_core_inputs(q, thickness, roughness, sld):
    """Host-side O(B*(Q+L)) prep; returns per-core input dicts."""
    q = q.astype(f32); th = thickness.astype(f32)
    rg = roughness.astype(f32); sld = sld.astype(f32)
    amb = sld[:, 0:1]
    negc64 = -(4.0 * math.pi * 1e-6) * (sld.astype(np.float64) - amb.astype(np.float64))
    negc = negc64.astype(f32)                       # [B, L+1]
    s2m16 = (-2.0 * rg * rg / (SC * SC)).astype(f16)   # scale folded
    t2 = (2.0 * th).astype(f32)
    m2t16 = (-2.0 * th / SC).astype(f16)               # scale folded
    qq = ((q * f32(0.5)) ** 2).astype(f32)          # [B, Q]

    def rep4(arr):  # [BL, K] -> [128, K] (each row repeated 4x)
        return np.repeat(arr, 4, axis=0).copy()

    in_maps = []
    for c in range(NCORES):
        bs = slice(c * BL, (c + 1) * BL)
        in_maps.append({
            "qq": qq[bs].reshape(P, QF).copy(),
            "negc": rep4(negc[bs]),
            "s2m16": rep4(s2m16[bs]),
            "t2": rep4(t2[bs]),
            "m2t16": rep4(m2t16[bs]),
        })
    return in_maps


def run(q, thickness, roughness, sld, trace=False, **trace_kwargs):
    from concourse.bass_utils import run_bass_kernel_spmd
    nc = _get_nc()
    in_maps = _prep_core_inputs(q, thickness, roughness, sld)
    res = run_bass_kernel_spmd(nc, in_maps, core_ids=list(range(NCORES)),
                               trace=trace, **trace_kwargs)
    out = np.empty((B, Q), f32)
    for c in range(NCORES):
        out[c * BL:(c + 1) * BL] = res.results[c]["out"].reshape(BL, Q)
    return out, res


def kernel(q, thickness, roughness, sld):
    out, _ = run(q, thickness, roughness, sld)
    return out


# revision 11
# speedup vs baseline: 3.3686x; 1.0415x over previous
"""Abeles matrix (neutron reflectivity) kernel for 8 Trainium2 NeuronCores.

Algorithm (per point (b,q), L=64 layers):
  X = (q/2)^2 - 4pi*(sld_l - sld_0)*1e-6, Y = 4pi*1e-9, R = sqrt(X^2+Y^2)
  k_l = A - iB: A = sqrt((R+X)/2) (stable for X>=0, clamped),
  B = min(sqrt((R-X)/2), (Y/2)/A) -- the min auto-selects the stable branch
  (clamp guarantees A_clamped <= A_true for X<0 so (Y/2)/A >= B_true there).
  r = (RDIF + 2i*CI0)/|S|^2 * exp(-2 s^2 PR)   [small-angle WI term dropped]
  E = exp(-2 t B) * (cos 2tA - i sin 2tA),  cos(x) = sin(pi/2 - |x|)
  scaled transfer recurrence (fp16): u0' = u0 + r*u1 ; u1' = E*(r*u0 + u1)
  out = |u1/u0|^2

fp16 bulk products are scaled by SC=2^8 (A,B scaled by SC) to avoid fp16
subnormal flush near the critical edge; the scale folds into ACT scale
factors, STT scalars and host-side param prep for free.

Sharding: pure data-parallel over batch, 32 rows of B=256 per core.
Per-core layout: 128 partitions = 32 b x 4 q-groups, 128 free = q within group.
Engine split: DVE = chain + fp16 2x bulk; ACT = transcendentals (table-set
grouped); GPSIMD = clamps, sums, scalar affine ops, negates.
"""
import sys
sys.path.insert(0, "/opt/trn_rl_repo")
import math
import numpy as np

import concourse.bass as bass
import concourse.mybir as mybir
from concourse import tile
from contextlib import ExitStack

AF = mybir.ActivationFunctionType
ALU = mybir.AluOpType
F32 = mybir.dt.float32
F16 = mybir.dt.float16
f32 = np.float32
f16 = np.float16

B, Q, L = 256, 512, 64
NCORES = 8
BL = B // NCORES           # 32 batch rows per core
P = 128                    # partitions
QF = 128                   # q elements per partition
CHUNK = 16                 # layers per chunk (4 chunks)

SC = 256.0                 # fp16 product scale (A,B scaled by SC)
ISC2 = f32(1.0 / (SC * SC))
YMAG = 4.0 * math.pi * 1e-9
Y2 = f32(YMAG * YMAG)
CLAMP = f32(4e-9)
LNYSC = f32(math.log(YMAG * SC / 2.0))
SC4 = f32(SC * SC * SC * SC)
ABIAS = f32(2e-9)
PIO2 = f32(np.pi / 2.0)
INV2PI = f32(1.0 / (2.0 * np.pi))
MAGIC = f32(1.5 * 2.0 ** 23)

# ---------------------------------------------------------------------------
# Toolchain workarounds for this walrus build:
# 1) InstDrain cannot carry sem waits -> re-emit as sync-engine wait_ge's.
# 2) TensorScalarPtr / Activation-with-AP-scale / CopyPredicated cannot carry
#    sem waits -> strip them onto same-engine wait_ge carrier instructions.
# ---------------------------------------------------------------------------
_PATCHED = False


def _install_patches():
    global _PATCHED
    if _PATCHED:
        return
    _PATCHED = True

    def _handles(tc):
        hm = {}
        for h in tc.sems.allocated().values():
            hm[h.name] = h
        return hm

    def _drain_and_barrier(self, tick_clock, wait_clock):
        nc = self.nc
        drain_inst = nc.sync.drain()
        wait_clock.add_sem_waits(
            drain_inst.ins, tile.ScopedClock({None: tick_clock.global_clock})
        )
        ii = drain_inst.ins
        si = ii.sync_info
        waits = list(si.on_wait) if si is not None else []
        if waits:
            ii.sync_info = mybir.SyncInfo(on_wait=[], on_update=list(si.on_update))
            hm = _handles(self)
            for w in waits:
                h = hm.get(w.ant_name)
                assert h is not None and w.wait_mode == "sem-ge-imm"
                nc.sync.wait_ge(h, w.wait_value)
        nc.all_engine_barrier()
        assert self.sems is not None
        popped = nc._tile_sem_poison_stack.pop()
        assert popped is self._sem_poison
        nc.clear_and_free_semaphores(list(self.sems.allocated().values()))
        nc.all_engine_barrier()

    tile.TileContext._drain_and_barrier = _drain_and_barrier

    _orig_commit = tile.TileContext._commit_instruction

    _KEEP1 = (mybir.InstTensorLoad, mybir.InstTensorSave, mybir.InstTensorCopy,
              mybir.InstTensorTensor)

    def _simple_aps(inst):
        # >2-dim APs lower to the S3S3D3-style structs with no wait slots
        try:
            for a in list(inst.ins) + list(inst.outs):
                ap = getattr(a, "ap", None)
                if ap is not None and len(ap) > 2:
                    return False
        except Exception:
            return False
        return True

    def _commit_instruction(self, inst, lazy_reg_writes=True):
        si = getattr(inst, "sync_info", None)
        if si is not None and si.on_wait:
            waits = list(si.on_wait)
            keep = []
            if isinstance(inst, _KEEP1) and _simple_aps(inst):
                # these structs tolerate one wait; strip the rest
                keep = waits[:1]
                waits = waits[1:]
            if waits:
                inst.sync_info = mybir.SyncInfo(on_wait=keep, on_update=list(si.on_update))
                hm = _handles(self)
                eng = self.nc.engines[inst.engine]
                for i in range(0, len(waits), 2):
                    grp = waits[i:i + 2]
                    h = hm.get(grp[0].ant_name)
                    assert h is not None and grp[0].wait_mode == "sem-ge-imm", grp
                    carrier = eng.wait_ge(h, grp[0].wait_value)
                    if len(grp) > 1:
                        csi = carrier.ins.sync_info
                        carrier.ins.sync_info = mybir.SyncInfo(
                            on_wait=list(grp),
                            on_update=list(csi.on_update) if csi else [])
        return _orig_commit(self, inst, lazy_reg_writes)

    tile.TileContext._commit_instruction = _commit_instruction


# ---------------------------------------------------------------------------
# Kernel builder (one NeuronCore program; SPMD across 8 cores)
# ---------------------------------------------------------------------------

def _build_kernel():
    _install_patches()
    nc = bass.Bass()

    d_qq = nc.declare_dram_parameter("qq", [P, QF], F32, isOutput=False)
    d_negc = nc.declare_dram_parameter("negc", [P, L + 1], F32, isOutput=False)
    d_s2mq = nc.declare_dram_parameter("s2mq", [P, L * QF], F16, isOutput=False)
    d_t2 = nc.declare_dram_parameter("t2", [P, L], F32, isOutput=False)
    d_m2tq = nc.declare_dram_parameter("m2tq", [P, L * QF], F16, isOutput=False)
    d_out = nc.declare_dram_parameter("out", [P, QF], F32, isOutput=True)

    with tile.TileContext(nc) as tc, ExitStack() as ctx:
        pool = ctx.enter_context(tc.tile_pool(name="sb", bufs=1))

        def tl(name, shape, dtype=F32, bufs=1):
            return pool.tile(shape, dtype, tag=name, name=name, bufs=bufs)

        # persistent inputs
        qq = tl("qq", [P, QF])
        negc = tl("negc", [P, L + 1])
        t2 = tl("t2", [P, L])
        nc.sync.dma_start(qq[:], d_qq[:])
        nc.sync.dma_start(negc[:], d_negc[:])
        nc.sync.dma_start(t2[:], d_t2[:])

        # constant bias vectors
        y2b = tl("y2b", [P, 1]);  nc.gpsimd.memset(y2b[:], float(Y2))
        nls2b = tl("nls2b", [P, 1]); nc.gpsimd.memset(nls2b[:], float(math.log(2.0) - math.log(SC * SC)))
        ab0 = tl("ab0", [P, 1]); nc.gpsimd.memset(ab0[:], 1e-16)
        lnyb = tl("lnyb", [P, 1]); nc.gpsimd.memset(lnyb[:], float(LNYSC))
        pio2b = tl("pio2b", [P, 1]); nc.gpsimd.memset(pio2b[:], float(PIO2))

        # u state (ping-pong quads: blocks [u0r | u0i | u1r | u1i]), fp16
        ucur = tl("uq_a", [P, 4 * QF], F16)
        unew = tl("uq_b", [P, 4 * QF], F16)
        nc.gpsimd.memset(ucur[:, 0:QF], 1.0)
        nc.gpsimd.memset(ucur[:, QF:], 0.0)
        A4 = tl("A4", [P, 4 * QF], F16); B4 = tl("B4", [P, 4 * QF], F16)
        T4 = tl("T4", [P, 4 * QF], F16)
        E4 = tl("E4", [P, 2 * QF], F16); E5 = tl("E5", [P, 2 * QF], F16)

        def pbc(param, lv0, n_l):
            # [P, n_l, QF] view of param[:, lv0:lv0+n_l], value broadcast over q
            return param[:, lv0:lv0 + n_l].rearrange("p (l n) -> p l n", n=1).broadcast_to([P, n_l, QF])

        def blk(t_, n_l):
            return t_[:, :n_l * QF].rearrange("p (l n) -> p l n", n=QF)

        prev_act = None   # serialization spine for ACT table-set ordering

        def act(out, in_, func, new_group=False, **kw):
            nonlocal prev_act
            i = nc.scalar.activation(out, in_, func, **kw)
            if prev_act is not None:
                bass._add_dep_helper(i.ins, prev_act.ins, sync=False,
                                     reason="act-table-order")
            prev_act = i
            return i

        starts = list(range(0, L, CHUNK))
        chunks = [(s0_, min(CHUNK, L - s0_)) for s0_ in starts]

        for l0, CL in reversed(chunks):
            cw = CL * QF
            cw1 = (CL + 1) * QF
            # ---- levels: X, R, U1, U2, A, B -------------------------------
            s2mf = tl("s2mf", [P, CHUNK * QF], F16)
            m2tf = tl("m2tf", [P, CHUNK * QF], F16)
            nc.sync.dma_start(s2mf[:, :cw], d_s2mq[:, l0 * QF:(l0 + CL) * QF])
            nc.sync.dma_start(m2tf[:, :cw], d_m2tq[:, l0 * QF:(l0 + CL) * QF])
            X = tl("X", [P, (CHUNK + 1) * QF])    # X -> U2 (in-place)
            SQ = tl("SQ", [P, (CHUNK + 1) * QF])  # SQ -> U1 -> LNA
            R = tl("Rr", [P, (CHUNK + 1) * QF])
            A32 = tl("A32", [P, (CHUNK + 1) * QF])
            A16 = tl("A16", [P, (CHUNK + 1) * QF], F16)
            B16 = tl("B16", [P, (CHUNK + 1) * QF], F16)
            BX16 = tl("BX16", [P, (CHUNK + 1) * QF], F16)
            MSKB = tl("MSKB", [P, (CHUNK + 1) * QF], F16)
            qqbc = qq[:].rearrange("p (l n) -> p l n", l=1).broadcast_to([P, CL + 1, QF])
            nc.vector.tensor_add(blk(X, CL + 1), qqbc, pbc(negc, l0, CL + 1))
            act(SQ[:, :cw1], X[:, :cw1], AF.Square)                          # [sqrt set]
            act(R[:, :cw1], SQ[:, :cw1], AF.Sqrt, bias=y2b[:])
            U1 = SQ  # SQ dead
            U2 = X   # X dies after MSK + U2 below
            nc.vector.tensor_scalar(MSKB[:, :cw1], X[:, :cw1], 0.0, None, ALU.is_ge)
            nc.vector.tensor_add(U1[:, :cw1], R[:, :cw1], X[:, :cw1])
            nc.vector.tensor_sub(U2[:, :cw1], R[:, :cw1], X[:, :cw1])        # in-place on X
            # clamps: subtract noise floor, floor at 0 (exact 0 for the dead branch)
            nc.vector.tensor_scalar(U1[:, :cw1], U1[:, :cw1], float(CLAMP), 0.0,
                                    ALU.subtract, ALU.max)
            nc.vector.tensor_scalar(U2[:, :cw1], U2[:, :cw1], float(CLAMP), 0.0,
                                    ALU.subtract, ALU.max)
            act(A32[:, :cw1], U1[:, :cw1], AF.Sqrt, bias=ab0[:], scale=0.5)
            act(A16[:, :cw1], U1[:, :cw1], AF.Sqrt, bias=0.0, scale=float(0.5 * SC * SC))
            act(B16[:, :cw1], U2[:, :cw1], AF.Sqrt, bias=0.0, scale=float(0.5 * SC * SC))
            # [logexp set] Bx = (Y*SC/2)/A  (stable for X>=0); merged additively:
            # B = Braw + (X>=0)*Bx  (A32 bias 1e-16 bounds LNA => Bx finite)
            LNA = U1  # U1 dead after sqrts
            act(LNA[:, :cw1], A32[:, :cw1], AF.Ln)
            act(BX16[:, :cw1], LNA[:, :cw1], AF.Exp, bias=lnyb[:], scale=-1.0)
            nc.vector.tensor_mul(BX16[:, :cw1], MSKB[:, :cw1], BX16[:, :cw1])
            nc.vector.tensor_add(B16[:, :cw1], B16[:, :cw1], BX16[:, :cw1])

            ac, an = A16[:, 0:cw], A16[:, QF:cw1]
            bc, bn = B16[:, 0:cw], B16[:, QF:cw1]
            rc, rn = R[:, 0:cw], R[:, QF:cw1]

            # ---- layer quantities ----------------------------------------
            P1 = tl("P1", [P, CHUNK * QF], F16)     # P1 -> PR -> RR16
            P2 = tl("P2", [P, CHUNK * QF], F16)     # P2 -> ARG
            AB1 = tl("AB1", [P, CHUNK * QF], F16)   # AB1 -> CI0
            AB2 = tl("AB2", [P, CHUNK * QF], F16)   # AB2 -> RATIM
            PP = tl("PP", [P, CHUNK * QF], F16)
            MAG = tl("MAG", [P, CHUNK * QF])        # MAG -> LNM (in-place)
            RSUM = tl("RSUM", [P, CHUNK * QF])      # RSUM -> ARG2
            RDIF = tl("RDIF", [P, CHUNK * QF])
            TA2 = tl("TA2", [P, CHUNK * QF])        # -> TA2r in-place
            FT = tl("FT", [P, CHUNK * QF])          # Ft -> KK in-place -> AbsT
            TB2 = tl("TB2", [P, CHUNK * QF], F16)
            EE = tl("EE", [P, CHUNK * QF], F16)
            S2T = tl("S2T", [P, CHUNK * QF], F16)
            C2T = tl("C2T", [P, CHUNK * QF], F16)
            RRE = tl("RRE", [P, CHUNK * QF], F16, bufs=2)
            RIMPM = tl("RIMPM", [P, 2 * CHUNK * QF], F16, bufs=2)
            ERE = tl("ERE", [P, CHUNK * QF], F16, bufs=2)
            EIMPM = tl("EIMPM", [P, 2 * CHUNK * QF], F16, bufs=2)
            rimv = RIMPM[:, :2 * cw].rearrange("p (l t n) -> p l t n", t=2, n=QF)
            eimv = EIMPM[:, :2 * cw].rearrange("p (l t n) -> p l t n", t=2, n=QF)

            nc.vector.tensor_mul(P1[:, :cw], ac, an)
            nc.vector.tensor_mul(P2[:, :cw], bc, bn)
            nc.vector.tensor_mul(AB1[:, :cw], ac, bn)
            nc.vector.tensor_mul(AB2[:, :cw], bc, an)
            nc.vector.tensor_add(PP[:, :cw], P1[:, :cw], P2[:, :cw])
            PR = P1   # in-place: P1 dead after PP
            nc.vector.tensor_sub(PR[:, :cw], P1[:, :cw], P2[:, :cw])
            CI0 = AB1  # in-place: AB1 dead after
            nc.vector.tensor_sub(CI0[:, :cw], AB1[:, :cw], AB2[:, :cw])
            nc.vector.tensor_add(RSUM[:, :cw], rc, rn)
            nc.vector.tensor_sub(RDIF[:, :cw], rc, rn)
            # MAG = |S|^2 unscaled f32; -ln(SC^2) folds into the Exp bias below
            nc.vector.scalar_tensor_tensor(MAG[:, :cw], PP[:, :cw], float(2.0 * ISC2),
                                           RSUM[:, :cw], ALU.mult, ALU.add)
            LNM = MAG
            act(LNM[:, :cw], MAG[:, :cw], AF.Ln)                              # in-place f32
            ARG = P2   # P2 dead after PP/PR
            nc.vector.tensor_mul(ARG[:, :cw], PR[:, :cw], s2mf[:, :cw])
            ARG2 = RSUM  # RSUM dead after MAG
            nc.vector.tensor_sub(ARG2[:, :cw], ARG[:, :cw], LNM[:, :cw])      # mixed -> f32
            RR16 = PR   # PR dead after ARG (fp16 slot); RR2 = 2*RR/SC^2
            act(RR16[:, :cw], ARG2[:, :cw], AF.Exp, bias=nls2b[:])
            nc.vector.scalar_tensor_tensor(RRE[:, :cw], RDIF[:, :cw], float(0.5 * SC * SC),
                                           RR16[:, :cw], ALU.mult, ALU.mult)  # RATRE fp16
            # RATIM = CI0 * RR2 (flat fp16 2x), then ACT copies into +/- slots
            RATIM = AB2  # AB2 dead after CI0
            nc.vector.tensor_mul(RATIM[:, :cw], CI0[:, :cw], RR16[:, :cw])
            act(rimv[:, :, 1, :], blk(RATIM, CL), AF.Copy, bias=0.0)
            act(rimv[:, :, 0, :], blk(RATIM, CL), AF.Copy, bias=0.0, scale=-1.0)
            # ---- E = exp(-2tB) * (cos 2tA - i sin 2tA) --------------------
            nc.vector.tensor_mul(blk(TA2, CL), blk(A32, CL), pbc(t2, l0, CL))
            nc.vector.tensor_mul(TB2[:, :cw], B16[:, :cw], m2tf[:, :cw])
            act(FT[:, :cw], TA2[:, :cw], AF.Copy, bias=float(MAGIC), scale=float(INV2PI))
            act(FT[:, :cw], FT[:, :cw], AF.Copy, bias=float(-MAGIC))             # KK in-place
            nc.vector.scalar_tensor_tensor(TA2[:, :cw], FT[:, :cw],
                                           float(-2.0 * np.pi), TA2[:, :cw],
                                           ALU.mult, ALU.add)                  # TA2r in-place
            act(EE[:, :cw], TB2[:, :cw], AF.Exp)
            act(FT[:, :cw], TA2[:, :cw], AF.Abs)                               # AbsT (FT dead)
            act(S2T[:, :cw], TA2[:, :cw], AF.Sin)                              # [trig set]
            act(C2T[:, :cw], FT[:, :cw], AF.Sin, bias=pio2b[:], scale=-1.0)
            nc.vector.tensor_mul(ERE[:, :cw], EE[:, :cw], C2T[:, :cw])
            nc.vector.tensor_mul(eimv[:, :, 0, :], blk(EE, CL), blk(S2T, CL))
            act(eimv[:, :, 1, :], eimv[:, :, 0, :], AF.Copy, bias=0.0, scale=-1.0)

            # ---- sequential update over layers (descending), fp16 ---------
            for j in range(CL - 1, -1, -1):
                sl = slice(j * QF, (j + 1) * QF)
                sl2 = slice(j * 2 * QF, (j + 1) * 2 * QF)
                U = ucur; Vq = unew
                U22 = U[:].rearrange("p (a b n) -> p a b n", a=2, b=2)
                u_rot = U22[:, ::-1, :, :]          # [u1r,u1i,u0r,u0i]
                u_rev = U22[:, ::-1, ::-1, :]       # [u1i,u1r,u0i,u0r]
                rre4 = RRE[:, sl].rearrange("p (a b n) -> p a b n", a=1, b=1).broadcast_to([P, 2, 2, QF])
                rim4 = RIMPM[:, sl2].rearrange("p (a t n) -> p a t n", a=1, t=2).broadcast_to([P, 2, 2, QF])
                nc.vector.tensor_mul(A4[:].rearrange("p (a b n) -> p a b n", a=2, b=2), u_rot, rre4)
                nc.vector.tensor_mul(B4[:].rearrange("p (a b n) -> p a b n", a=2, b=2), u_rev, rim4)
                nc.vector.tensor_add(T4[:], U[:], A4[:])
                nc.vector.tensor_add(Vq[:], T4[:], B4[:])   # [nu0r, nu0i, p1r, p1i]
                p1 = Vq[:, 2 * QF:4 * QF]
                p12 = p1.rearrange("p (b n) -> p b n", b=2)
                p1sw = p12[:, ::-1, :]
                ere2 = ERE[:, sl].rearrange("p (b n) -> p b n", b=1).broadcast_to([P, 2, QF])
                nc.vector.tensor_mul(E4[:].rearrange("p (b n) -> p b n", b=2), p12, ere2)
                nc.vector.tensor_mul(E5[:].rearrange("p (b n) -> p b n", b=2), p1sw,
                                     EIMPM[:, sl2].rearrange("p (b n) -> p b n", b=2))
                nc.vector.tensor_add(Vq[:, 2 * QF:4 * QF], E4[:], E5[:])
                ucur, unew = unew, ucur

        # ---- epilogue: out = |u1/u0|^2 -----------------------------------
        u0r, u0i = ucur[:, 0:QF], ucur[:, QF:2 * QF]
        u1r, u1i = ucur[:, 2 * QF:3 * QF], ucur[:, 3 * QF:4 * QF]
        d1 = tl("q1", [P, QF]); d2 = tl("q2", [P, QF]); d3 = tl("q3", [P, QF])
        act(d1[:], u0r, AF.Square)
        act(d2[:], u0i, AF.Square)
        nc.vector.tensor_add(d1[:], d1[:], d2[:])      # |u0|^2
        act(d2[:], d1[:], AF.Ln)
        act(d1[:], d2[:], AF.Exp, bias=0.0, scale=-1.0)  # 1/|u0|^2
        nc.vector.tensor_mul(d2[:], u1r, u0r)
        nc.vector.tensor_mul(d3[:], u1i, u0i)
        nc.vector.tensor_add(d2[:], d2[:], d3[:])
        nc.vector.tensor_mul(d2[:], d2[:], d1[:])      # qr
        OUT = tl("OUT", [P, QF])
        nc.vector.tensor_mul(d3[:], u1i, u0r)
        qi2 = tl("q4", [P, QF])
        nc.vector.tensor_mul(qi2[:], u1r, u0i)
        nc.vector.tensor_sub(d3[:], d3[:], qi2[:])
        nc.vector.tensor_mul(d3[:], d3[:], d1[:])      # qi
        act(d2[:], d2[:], AF.Square)
        act(d3[:], d3[:], AF.Square)
        nc.vector.tensor_add(OUT[:], d2[:], d3[:])
        nc.sync.dma_start(d_out[:], OUT[:])

    return nc


_NC_CACHE = None


def _get_nc():
    global _NC_CACHE
    if _NC_CACHE is None:
        _NC_CACHE = _build_kernel()
    return _NC_CACHE


def _prep_core_inputs(q, thickness, roughness, sld):
    """Host-side O(B*(Q+L)) prep; returns per-core input dicts."""
    q = q.astype(f32); th = thickness.astype(f32)
    rg = roughness.astype(f32); sld = sld.astype(f32)
    amb = sld[:, 0:1]
    negc64 = -(4.0 * math.pi * 1e-6) * (sld.astype(np.float64) - amb.astype(np.float64))
    negc = negc64.astype(f32)                       # [B, L+1]
    s2m16 = (-2.0 * rg * rg / (SC * SC)).astype(f16)   # scale folded
    t2 = (2.0 * th).astype(f32)
    m2t16 = (-2.0 * th / SC).astype(f16)               # scale folded
    qq = ((q * f32(0.5)) ** 2).astype(f32)          # [B, Q]

    def rep4(arr):  # [BL, K] -> [128, K] (each row repeated 4x)
        return np.repeat(arr, 4, axis=0).copy()

    in_maps = []
    for c in range(NCORES):
        bs = slice(c * BL, (c + 1) * BL)
        in_maps.append({
            "qq": qq[bs].reshape(P, QF).copy(),
            "negc": rep4(negc[bs]),
            "s2mq": np.repeat(rep4(s2m16[bs]), QF, axis=1).copy(),
            "t2": rep4(t2[bs]),
            "m2tq": np.repeat(rep4(m2t16[bs]), QF, axis=1).copy(),
        })
    return in_maps


def run(q, thickness, roughness, sld, trace=False, **trace_kwargs):
    from concourse.bass_utils import run_bass_kernel_spmd
    nc = _get_nc()
    in_maps = _prep_core_inputs(q, thickness, roughness, sld)
    res = run_bass_kernel_spmd(nc, in_maps, core_ids=list(range(NCORES)),
                               trace=trace, **trace_kwargs)
    out = np.empty((B, Q), f32)
    for c in range(NCORES):
        out[c * BL:(c + 1) * BL] = res.results[c]["out"].reshape(BL, Q)
    return out, res


def kernel(q, thickness, roughness, sld):
    out, _ = run(q, thickness, roughness, sld)
    return out
